# revision 8
# baseline (speedup 1.0000x reference)
"""Trainium2 Bass kernel for nn_ComplexSuperposition.

Math (per batch b):
    or = sum_t w[b,t] * x_r[b,t,:]          # [D]
    oi = sum_t w[b,t] * x_i[b,t,:]          # [D]
    out_r[b] = or (x) or + oi (x) oi        # [D,D]
    out_i[b] = oi (x) or - or (x) oi        # [D,D]

Strategy: pure data-parallel over B=128 across 8 cores (16 batches/core),
fp16 operands, fp16 DRAM outputs (upcast on host), block-upper-triangle
outputs (out_r symmetric, out_i antisymmetric; PE products are exactly
mirror-consistent, so the host mirror adds zero error).

  Phase A (per pair of batches): weighted sums as K=T matmuls with a
    host-precomputed lhsT bundle `wxa`: for each batch, 4 matmuls of
    N=512 build PSUM rows (or, oi) in bank-plane 0 and (-oi, or) in
    bank-plane 1 (even batch rows 0-1, odd batch rows 32-33 => row
    groups 0/1 for phase B). One cast-copy evacuates all rows to SBUF
    fp16 (`mvst`).
  Phase B (per batch): rank-2 outer products out = lhsT.T @ rhs with
    K=2.  out_r chunk m = mv[:,msl].T @ mv[:,nsl]; out_i chunk m =
    mv[:,msl].T @ st'[:,nsl] with st' = (-oi, or) -- same stationary
    operand for both planes.  The 4 triangle chunks of each plane are
    packed into 3 PSUM banks (m0 | m1+m3 | m2) so evacuation is two
    wide copies per plane (r-plane on Vector, i-plane on Scalar).
    Even/odd batches alternate PE row groups 0/1 so LDWEIGHTS overlaps
    in-flight matmuls.

CS_HOSTW=1 moves the (cheap, 20% of MACs) weighted sums to the host:
the device then only loads the tiny (or,oi) bundles and computes +
stores the outer products (the remaining 80% of MACs and ~all bytes).

DRAM layout: out_r/out_i [B_LOC, 128, 1280] fp16, chunk m of row block
m at free offset TRI_OFF[m] = (0, 512, 1024, 896); host mirrors the
lower blocks (sym=+1 real / sym=-1 imag).
"""

import os
from contextlib import ExitStack

import numpy as np

N_CORES = 8
B, T, D = 128, 128, 512
B_LOC = B // N_CORES  # 16
NPAIR = B_LOC // 2    # 8

HOSTW = os.environ.get("CS_HOSTW", "0") == "1"
TRI_OFF = (0, 512, 1024, 896)  # free-dim offset of chunk m in packed row
TRI_W = 1280

_CACHE = {}


def _build_program():
    import concourse.bacc as bacc
    import concourse.tile as tile
    from concourse import mybir

    f32 = mybir.dt.float32
    f16 = mybir.dt.float16

    nc = bacc.Bacc("TRN2", target_bir_lowering=False, debug=False)

    if not HOSTW:
        xr_d = nc.dram_tensor("input_real", [B_LOC, T, D], f16, kind="ExternalInput").ap()
        xi_d = nc.dram_tensor("input_imag", [B_LOC, T, D], f16, kind="ExternalInput").ap()
        wxa_d = nc.dram_tensor("wxa", [T, 80 * NPAIR], f16, kind="ExternalInput").ap()
    else:
        # host-computed (or, oi, -oi, or) bundles: [4 rows, plane, pair, D]
        wm_d = nc.dram_tensor("wm", [4, 2, NPAIR, D], f16, kind="ExternalInput").ap()
    or_d = nc.dram_tensor("out_r", [B_LOC, 128, TRI_W], f16, kind="ExternalOutput").ap()
    oi_d = nc.dram_tensor("out_i", [B_LOC, 128, TRI_W], f16, kind="ExternalOutput").ap()

    with tile.TileContext(nc) as tc, ExitStack() as ctx:
        singles = ctx.enter_context(tc.tile_pool(name="singles", bufs=1))
        if not HOSTW:
            xpool = ctx.enter_context(tc.tile_pool(name="x", bufs=16))
            vpool = ctx.enter_context(tc.tile_pool(name="vec", bufs=6))
        opool = ctx.enter_context(tc.tile_pool(name="outs", bufs=6))
        if not HOSTW:
            psa = ctx.enter_context(tc.tile_pool(name="psa", bufs=1, space="PSUM"))
        psb = ctx.enter_context(tc.tile_pool(name="psb", bufs=2, space="PSUM"))

        if not HOSTW:
            wxa = singles.tile([T, 80 * NPAIR], f16)
            nc.sync.dma_start(out=wxa[:], in_=wxa_d[:])
        else:
            mvall = singles.tile([34, 2, NPAIR, D], f16)
            nc.sync.dma_start(out=mvall[0:2], in_=wm_d[0:2])
            nc.sync.dma_start(out=mvall[32:34], in_=wm_d[2:4])

        # PE warmup: dense tiny matmuls during the load prologue so the
        # HAM clock gate reaches 8/8 before the real matmuls start.
        warm = singles.tile([2, 64], f16)
        nc.gpsimd.memset(warm[:], 0)
        wps = psb.tile([128, 512], f32, tag="tA")
        for _ in range(40):
            nc.tensor.matmul(wps[:32, :64], lhsT=warm[:, :32], rhs=warm[:], start=True, stop=True)

        for p in range(NPAIR):
            c0 = 2 * p

            if not HOSTW:
                xr01 = xpool.tile([T, 2, D], f16, tag="x")
                nc.gpsimd.dma_start(out=xr01[:], in_=xr_d[c0 : c0 + 2].rearrange("j t d -> t j d"))
                xi01 = xpool.tile([T, 2, D], f16, tag="x")
                nc.gpsimd.dma_start(out=xi01[:], in_=xi_d[c0 : c0 + 2].rearrange("j t d -> t j d"))

                # Phase A into one 2-bank tile: plane 0 rows (0,1 | 32,33) =
                # (or,oi) even|odd, plane 1 = (-oi, or) even|odd.  The first
                # matmul of each plane is M=34 (one hot column + zeros) so the
                # whole partition range of the accumulation group is written.
                pa = psa.tile([34, 2, D], f32, tag="pa")
                o = 80 * p
                xr0_, xr1_ = xr01[:, 0, :], xr01[:, 1, :]
                xi0_, xi1_ = xi01[:, 0, :], xi01[:, 1, :]
                nc.tensor.matmul(pa[0:34, 0, :], lhsT=wxa[:, o : o + 34], rhs=xr0_[:], start=True, stop=False, skip_group_check=True)
                nc.tensor.matmul(pa[0:34, 1, :], lhsT=wxa[:, o + 40 : o + 74], rhs=xi0_[:], start=True, stop=False, skip_group_check=True)
                nc.tensor.matmul(pa[0:2, 0, :], lhsT=wxa[:, o + 34 : o + 36], rhs=xi0_[:], start=False, stop=False, skip_group_check=True)
                nc.tensor.matmul(pa[0:2, 1, :], lhsT=wxa[:, o + 74 : o + 76], rhs=xr0_[:], start=False, stop=False, skip_group_check=True)
                nc.tensor.matmul(pa[32:34, 0, :], lhsT=wxa[:, o + 36 : o + 38], rhs=xr1_[:], start=False, stop=False, skip_group_check=True)
                nc.tensor.matmul(pa[32:34, 1, :], lhsT=wxa[:, o + 76 : o + 78], rhs=xi1_[:], start=False, stop=False, skip_group_check=True)
                nc.tensor.matmul(pa[32:34, 0, :], lhsT=wxa[:, o + 38 : o + 40], rhs=xi1_[:], start=False, stop=True, skip_group_check=True)
                nc.tensor.matmul(pa[32:34, 1, :], lhsT=wxa[:, o + 78 : o + 80], rhs=xr1_[:], start=False, stop=True, skip_group_check=True)

                # Evacuate each batch's (mv, st') rows separately so phase B
                # of batch j only waits on its own copy.
                mvst = vpool.tile([34, 2, D], f16, tag="op")
                nc.vector.tensor_copy(out=mvst[0:2], in_=pa[0:2])
                nc.scalar.copy(out=mvst[32:34], in_=pa[32:34])

            big = opool.tile([128, 4, TRI_W], f16, tag="big")  # planes: r_e, i_e, r_o, i_o
            for j in (0, 1):
                r0 = 32 * j
                if HOSTW:
                    mv = mvall[r0 : r0 + 2, 0, p, :]
                    st = mvall[r0 : r0 + 2, 1, p, :]
                else:
                    mv = mvst[r0 : r0 + 2, 0, :]
                    st = mvst[r0 : r0 + 2, 1, :]
                # 3-bank packed chunk layout per plane:
                #   tA[0:512]    = m0
                #   tB[0:384]    = m1, tB[384:512] = m3, tB[512:768] = m2
                tAr = psb.tile([128, 512], f32, tag="tA")
                tBr = psb.tile([128, 1024], f32, tag="tB")
                tAi = psb.tile([128, 512], f32, tag="tA")
                tBi = psb.tile([128, 1024], f32, tag="tB")
                # (target_r, target_i, start, stop): m1+m3 share tB bank 0 as
                # one accumulation group (disjoint regions -> overwrite), m2
                # alone in tB bank 1, m0 alone in tA.
                plan = [
                    (0, tAr[:, 0:512], tAi[:, 0:512], True, True),
                    (1, tBr[:, 0:384], tBi[:, 0:384], True, False),
                    (3, tBr[:, 384:512], tBi[:, 384:512], False, True),
                    (2, tBr[:, 512:768], tBi[:, 512:768], True, True),
                ]
                for m, tr, ti, st_, sp_ in plan:
                    msl = slice(128 * m, 128 * m + 128)
                    nsl = slice(128 * m, D)
                    nc.tensor.matmul(tr, lhsT=mv[:, msl], rhs=mv[:, nsl], start=st_, stop=sp_, skip_group_check=True)
                    nc.tensor.matmul(ti, lhsT=mv[:, msl], rhs=st[:, nsl], start=st_, stop=sp_, skip_group_check=True)
                # evac: r-plane on Vector, i-plane on Scalar
                pr, pi = 2 * j, 2 * j + 1
                nc.vector.tensor_copy(out=big[:, pr, 0:512], in_=tAr[:, :])
                nc.vector.tensor_copy(out=big[:, pr, 512:1280], in_=tBr[:, 0:768])
                nc.scalar.copy(out=big[:, pi, 0:512], in_=tAi[:, :])
                nc.scalar.copy(out=big[:, pi, 512:1280], in_=tBi[:, 0:768])

            bgr = big[:].rearrange("p (b j) n -> p b j n", j=2)
            if p in (0, NPAIR - 1):
                # first/last pair: per-batch DMAs to shorten pipeline fill
                # and drain
                for jb in (0, 1):
                    nc.sync.dma_start(out=or_d[c0 + jb], in_=bgr[:, jb, 0, :])
                    nc.sync.dma_start(out=oi_d[c0 + jb], in_=bgr[:, jb, 1, :])
            else:
                nc.sync.dma_start(
                    out=or_d[c0 : c0 + 2].rearrange("b p n -> p b n"),
                    in_=bgr[:, :, 0, :],
                )
                nc.sync.dma_start(
                    out=oi_d[c0 : c0 + 2].rearrange("b p n -> p b n"),
                    in_=bgr[:, :, 1, :],
                )

    nc.compile()
    return nc


def _get_nc():
    if "nc" not in _CACHE:
        _CACHE["nc"] = _build_program()
    return _CACHE["nc"]


def _make_in_maps(input_real, input_imag, weight):
    in_maps = []
    if not HOSTW:
        for core in range(N_CORES):
            sl = slice(core * B_LOC, (core + 1) * B_LOC)
            wc = weight[sl]  # [B_LOC, T]
            wxa = np.zeros((T, 80 * NPAIR), np.float32)
            for p in range(NPAIR):
                o = 80 * p
                we, wo = wc[2 * p], wc[2 * p + 1]
                # plane0 block [o:o+40]:  M=34 (col0=w_e), then (0,w_e),
                # (w_o,0), (0,w_o); plane1 block [o+40:o+80]: M=34
                # (col0=-w_e), then (0,w_e), (-w_o,0), (0,w_o)
                wxa[:, o + 0] = we       # pl0 rhs=xr0 -> row0 = or_e
                wxa[:, o + 35] = we      # pl0 rhs=xi0 -> row1 = oi_e
                wxa[:, o + 36] = wo      # pl0 rhs=xr1 -> row32 = or_o
                wxa[:, o + 39] = wo      # pl0 rhs=xi1 -> row33 = oi_o
                wxa[:, o + 40] = -we     # pl1 rhs=xi0 -> row0 = -oi_e
                wxa[:, o + 75] = we      # pl1 rhs=xr0 -> row1 = or_e
                wxa[:, o + 76] = -wo     # pl1 rhs=xi1 -> row32 = -oi_o
                wxa[:, o + 79] = wo      # pl1 rhs=xr1 -> row33 = or_o
            in_maps.append(
                {
                    "input_real": np.ascontiguousarray(input_real[sl], dtype=np.float16),
                    "input_imag": np.ascontiguousarray(input_imag[sl], dtype=np.float16),
                    "wxa": np.ascontiguousarray(wxa, dtype=np.float16),
                }
            )
    else:
        xr16 = input_real.astype(np.float16).astype(np.float32)
        xi16 = input_imag.astype(np.float16).astype(np.float32)
        orv = np.einsum("btd,bt->bd", xr16, weight)
        oiv = np.einsum("btd,bt->bd", xi16, weight)
        for core in range(N_CORES):
            sl = slice(core * B_LOC, (core + 1) * B_LOC)
            oc_r = orv[sl].reshape(NPAIR, 2, D)
            oc_i = oiv[sl].reshape(NPAIR, 2, D)
            wm = np.empty((4, 2, NPAIR, D), np.float32)
            for j in (0, 1):
                wm[2 * j + 0, 0] = oc_r[:, j]   # mv row 0 = or
                wm[2 * j + 1, 0] = oc_i[:, j]   # mv row 1 = oi
                wm[2 * j + 0, 1] = -oc_i[:, j]  # st' row 0 = -oi
                wm[2 * j + 1, 1] = oc_r[:, j]   # st' row 1 = or
            in_maps.append({"wm": np.ascontiguousarray(wm, dtype=np.float16)})
    return in_maps


def _expand_tri(tri, sym):
    """tri: [B, 128, 1280] packed block-upper rows -> full [B, D, D].
    Chunk m holds rows [128m,128m+128) x cols [128m, D). Lower blocks are
    mirrored (sym=+1) or negated-mirrored (sym=-1)."""
    Bn = tri.shape[0]
    full = np.empty((Bn, D, D), dtype=np.float32)
    for m in range(4):
        rs = slice(m * 128, (m + 1) * 128)
        full[:, rs, m * 128 :] = tri[:, :, TRI_OFF[m] : TRI_OFF[m] + D - m * 128]
    for m in range(4):
        for n in range(m):
            full[:, m * 128 : (m + 1) * 128, n * 128 : (n + 1) * 128] = (
                sym * full[:, n * 128 : (n + 1) * 128, m * 128 : (m + 1) * 128]
                .transpose(0, 2, 1)
            )
    return full


def run(input_real, input_imag, weight, trace=False, **spmd_kwargs):
    """Build+run; returns (out_r, out_i, BassKernelResults)."""
    from concourse.bass_utils import run_bass_kernel_spmd

    input_real = np.asarray(input_real, dtype=np.float32)
    input_imag = np.asarray(input_imag, dtype=np.float32)
    weight = np.asarray(weight, dtype=np.float32)
    assert input_real.shape == (B, T, D), input_real.shape
    assert weight.shape == (B, T), weight.shape

    nc = _get_nc()
    in_maps = _make_in_maps(input_real, input_imag, weight)
    res = run_bass_kernel_spmd(
        nc, in_maps, list(range(N_CORES)), trace=trace, **spmd_kwargs
    )
    tri_r = np.concatenate([np.asarray(r["out_r"]) for r in res.results], axis=0)
    tri_i = np.concatenate([np.asarray(r["out_i"]) for r in res.results], axis=0)
    out_r = _expand_tri(tri_r, sym=1.0)
    out_i = _expand_tri(tri_i, sym=-1.0)
    return out_r, out_i, res


def kernel(input_real, input_imag, weight):
    out_r, out_i, _ = run(input_real, input_imag, weight)
    return out_r, out_i


# revision 16
# speedup vs baseline: 1.0465x; 1.0465x over previous
"""Trainium2 Bass kernel for nn_ComplexSuperposition.

Math (per batch b):
    or = sum_t w[b,t] * x_r[b,t,:]          # [D]
    oi = sum_t w[b,t] * x_i[b,t,:]          # [D]
    out_r[b] = or (x) or + oi (x) oi        # [D,D]
    out_i[b] = oi (x) or - or (x) oi        # [D,D]

Strategy: pure data-parallel over B=128 across 8 cores (16 batches/core),
fp16 operands, fp16 DRAM outputs (upcast on host), block-upper-triangle
outputs (out_r symmetric, out_i antisymmetric; PE products are exactly
mirror-consistent, so the host mirror adds zero error).

  Phase A (per pair of batches): weighted sums as K=T matmuls with a
    host-precomputed lhsT bundle `wxa`: for each batch, 4 matmuls of
    N=512 build PSUM rows (or, oi) in bank-plane 0 and (-oi, or) in
    bank-plane 1 (even batch rows 0-1, odd batch rows 32-33 => row
    groups 0/1 for phase B). One cast-copy evacuates all rows to SBUF
    fp16 (`mvst`).
  Phase B (per batch): rank-2 outer products out = lhsT.T @ rhs with
    K=2.  out_r chunk m = mv[:,msl].T @ mv[:,nsl]; out_i chunk m =
    mv[:,msl].T @ st'[:,nsl] with st' = (-oi, or) -- same stationary
    operand for both planes.  The 4 triangle chunks of each plane are
    packed into 3 PSUM banks (m0 | m1+m3 | m2) so evacuation is two
    wide copies per plane (r-plane on Vector, i-plane on Scalar).
    Even/odd batches alternate PE row groups 0/1 so LDWEIGHTS overlaps
    in-flight matmuls.

CS_HOSTW=1 moves the (cheap, 20% of MACs) weighted sums to the host:
the device then only loads the tiny (or,oi) bundles and computes +
stores the outer products (the remaining 80% of MACs and ~all bytes).

DRAM layout: out_r/out_i [B_LOC, 128, 1280] fp16, chunk m of row block
m at free offset TRI_OFF[m] = (0, 512, 1024, 896); host mirrors the
lower blocks (sym=+1 real / sym=-1 imag).
"""

import os
from contextlib import ExitStack

import numpy as np

N_CORES = 8
B, T, D = 128, 128, 512
B_LOC = B // N_CORES  # 16
NPAIR = B_LOC // 2    # 8

HOSTW = os.environ.get("CS_HOSTW", "0") == "1"
TRI_OFF = (0, 512, 1024, 896)  # free-dim offset of chunk m in packed row
TRI_W = 1280

_CACHE = {}


def _build_program():
    import concourse.bacc as bacc
    import concourse.tile as tile
    from concourse import mybir

    f32 = mybir.dt.float32
    f16 = mybir.dt.float16

    nc = bacc.Bacc("TRN2", target_bir_lowering=False, debug=False)

    if not HOSTW:
        xr_d = nc.dram_tensor("input_real", [B_LOC, T, D], f16, kind="ExternalInput").ap()
        xi_d = nc.dram_tensor("input_imag", [B_LOC, T, D], f16, kind="ExternalInput").ap()
        wxa_d = nc.dram_tensor("wxa", [T, 400 * NPAIR], f16, kind="ExternalInput").ap()
    else:
        # host-computed (or, oi) / (-oi, or) bundles, duplicated per row
        # group: [group, row, plane, pair, D] -> sbuf partitions 32g..32g+1
        wm_d = nc.dram_tensor("wm", [4, 2, 2, NPAIR, D], f16, kind="ExternalInput").ap()
    or_d = nc.dram_tensor("out_r", [B_LOC, 128, TRI_W], f16, kind="ExternalOutput").ap()
    oi_d = nc.dram_tensor("out_i", [B_LOC, 128, TRI_W], f16, kind="ExternalOutput").ap()

    with tile.TileContext(nc) as tc, ExitStack() as ctx:
        singles = ctx.enter_context(tc.tile_pool(name="singles", bufs=1))
        if not HOSTW:
            xpool = ctx.enter_context(tc.tile_pool(name="x", bufs=16))
            vpool = ctx.enter_context(tc.tile_pool(name="vec", bufs=6))
        opool = ctx.enter_context(tc.tile_pool(name="outs", bufs=6))
        if not HOSTW:
            psa = ctx.enter_context(tc.tile_pool(name="psa", bufs=1, space="PSUM"))
        psb = ctx.enter_context(tc.tile_pool(name="psb", bufs=2, space="PSUM"))

        if not HOSTW:
            wxa = singles.tile([T, 400 * NPAIR], f16)
            nc.sync.dma_start(out=wxa[:], in_=wxa_d[:])
        else:
            mvall = singles.tile([98, 2, NPAIR, D], f16)
            for g in range(4):
                nc.sync.dma_start(out=mvall[32 * g : 32 * g + 2], in_=wm_d[g])

        # PE warmup: dense tiny matmuls during the load prologue so the
        # HAM clock gate reaches 8/8 before the real matmuls start.
        warm = singles.tile([2, 64], f16)
        nc.gpsimd.memset(warm[:], 0)
        wps = psb.tile([128, 512], f32, tag="tA")
        for _ in range(40):
            nc.tensor.matmul(wps[:32, :64], lhsT=warm[:, :32], rhs=warm[:], start=True, stop=True)

        for p in range(NPAIR):
            c0 = 2 * p

            if not HOSTW:
                xr01 = xpool.tile([T, 2, D], f16, tag="x")
                nc.gpsimd.dma_start(out=xr01[:], in_=xr_d[c0 : c0 + 2].rearrange("j t d -> t j d"))
                xi01 = xpool.tile([T, 2, D], f16, tag="x")
                nc.gpsimd.dma_start(out=xi01[:], in_=xi_d[c0 : c0 + 2].rearrange("j t d -> t j d"))

                # Phase A into one 2-bank tile.  Each batch's (or, oi) pair is
                # written TWICE (row groups 64j and 64j+32) so phase B can
                # alternate PE row groups between chunk matmuls.  Plane 0 =
                # mv = (or, oi), plane 1 = st' = (-oi, or).  The first matmul
                # of each plane is M=98 (one-hot columns + zeros) so the whole
                # partition range of the accumulation group is initialized.
                pa = psa.tile([98, 2, D], f32, tag="pa")
                o = 400 * p
                xr0_, xr1_ = xr01[:, 0, :], xr01[:, 1, :]
                xi0_, xi1_ = xi01[:, 0, :], xi01[:, 1, :]
                # widths: W1[98] W2[98] then 6x W[34]
                o3, o4, o5, o6, o7, o8 = (o + 196, o + 230, o + 264, o + 298, o + 332, o + 366)
                nc.tensor.matmul(pa[0:98, 0, :], lhsT=wxa[:, o : o + 98], rhs=xr0_[:], start=True, stop=False, skip_group_check=True)
                nc.tensor.matmul(pa[0:98, 1, :], lhsT=wxa[:, o + 98 : o + 196], rhs=xi0_[:], start=True, stop=False, skip_group_check=True)
                nc.tensor.matmul(pa[0:34, 0, :], lhsT=wxa[:, o3 : o3 + 34], rhs=xi0_[:], start=False, stop=False, skip_group_check=True)
                nc.tensor.matmul(pa[0:34, 1, :], lhsT=wxa[:, o4 : o4 + 34], rhs=xr0_[:], start=False, stop=False, skip_group_check=True)
                nc.tensor.matmul(pa[64:98, 0, :], lhsT=wxa[:, o5 : o5 + 34], rhs=xr1_[:], start=False, stop=False, skip_group_check=True)
                nc.tensor.matmul(pa[64:98, 1, :], lhsT=wxa[:, o6 : o6 + 34], rhs=xi1_[:], start=False, stop=False, skip_group_check=True)
                nc.tensor.matmul(pa[64:98, 0, :], lhsT=wxa[:, o7 : o7 + 34], rhs=xi1_[:], start=False, stop=True, skip_group_check=True)
                nc.tensor.matmul(pa[64:98, 1, :], lhsT=wxa[:, o8 : o8 + 34], rhs=xr1_[:], start=False, stop=True, skip_group_check=True)

                # Evacuate per batch and per plane (phase B r-matmuls of batch
                # j only wait on that batch's plane-0 copy).
                mvst = vpool.tile([98, 2, D], f16, tag="op")
                nc.vector.tensor_copy(out=mvst[0:34, 0], in_=pa[0:34, 0])
                nc.scalar.copy(out=mvst[64:98, 0], in_=pa[64:98, 0])
                nc.vector.tensor_copy(out=mvst[0:34, 1], in_=pa[0:34, 1])
                nc.scalar.copy(out=mvst[64:98, 1], in_=pa[64:98, 1])

            big = opool.tile([128, 4, TRI_W], f16, tag="big")  # planes: r_e, i_e, r_o, i_o

            def mv_(g):
                return mvall[g : g + 2, 0, p, :] if HOSTW else mvst[g : g + 2, 0, :]

            def st_(g):
                return mvall[g : g + 2, 1, p, :] if HOSTW else mvst[g : g + 2, 1, :]

            for j in (0, 1):
                gA, gB = 64 * j, 64 * j + 32
                # 3-bank packed chunk layout per plane:
                #   tA[0:512]    = m0
                #   tB[0:384]    = m1, tB[384:512] = m3, tB[512:768] = m2
                tAr = psb.tile([128, 512], f32, tag="tA")
                tBr = psb.tile([128, 1024], f32, tag="tB")
                tAi = psb.tile([128, 512], f32, tag="tA")
                tBi = psb.tile([128, 1024], f32, tag="tB")
                # (m, target, row group, start, stop, is_i): m1+m3 share tB
                # bank 0 as one accumulation group (disjoint regions ->
                # overwrite), m2 alone in tB bank 1, m0 alone in tA.
                # Matmuls sharing a PSUM bank MUST share a row group (two
                # concurrent row-group streams into one bank wedge the HW),
                # so m1/m3 are same-group; the r and i planes use opposite
                # group assignments and interleave so consecutive matmuls
                # still alternate row groups (LDWEIGHTS overlap + PE
                # concurrency).
                seq = [
                    (0, tAr[:, 0:512], gA, True, True, False),
                    (1, tBr[:, 0:384], gB, True, False, False),
                    (1, tBi[:, 0:384], gA, True, False, True),
                    (3, tBr[:, 384:512], gB, False, True, False),
                    (3, tBi[:, 384:512], gA, False, True, True),
                    (0, tAi[:, 0:512], gB, True, True, True),
                    (2, tBr[:, 512:768], gA, True, True, False),
                    (2, tBi[:, 512:768], gB, True, True, True),
                ]
                for m, tgt, g, st1, sp1, is_i in seq:
                    msl = slice(128 * m, 128 * m + 128)
                    nsl = slice(128 * m, D)
                    rhs = st_(g) if is_i else mv_(g)
                    nc.tensor.matmul(tgt, lhsT=mv_(g)[:, msl], rhs=rhs[:, nsl], start=st1, stop=sp1, skip_group_check=True, tile_position=(g, 0))
                # evac: r-plane on Vector, i-plane on Scalar
                pr, pi = 2 * j, 2 * j + 1
                nc.vector.tensor_copy(out=big[:, pr, 0:512], in_=tAr[:, :])
                nc.vector.tensor_copy(out=big[:, pr, 512:1280], in_=tBr[:, 0:768])
                nc.scalar.copy(out=big[:, pi, 0:512], in_=tAi[:, :])
                nc.scalar.copy(out=big[:, pi, 512:1280], in_=tBi[:, 0:768])

            bgr = big[:].rearrange("p (b j) n -> p b j n", j=2)
            if p in (0, NPAIR - 1):
                # first/last pair: per-batch DMAs to shorten pipeline fill
                # and drain
                for jb in (0, 1):
                    nc.sync.dma_start(out=or_d[c0 + jb], in_=bgr[:, jb, 0, :])
                    nc.sync.dma_start(out=oi_d[c0 + jb], in_=bgr[:, jb, 1, :])
            else:
                nc.sync.dma_start(
                    out=or_d[c0 : c0 + 2].rearrange("b p n -> p b n"),
                    in_=bgr[:, :, 0, :],
                )
                nc.sync.dma_start(
                    out=oi_d[c0 : c0 + 2].rearrange("b p n -> p b n"),
                    in_=bgr[:, :, 1, :],
                )

    nc.compile()
    return nc


def _get_nc():
    if "nc" not in _CACHE:
        _CACHE["nc"] = _build_program()
    return _CACHE["nc"]


def _make_in_maps(input_real, input_imag, weight):
    in_maps = []
    if not HOSTW:
        for core in range(N_CORES):
            sl = slice(core * B_LOC, (core + 1) * B_LOC)
            wc = weight[sl]  # [B_LOC, T]
            wxa = np.zeros((T, 400 * NPAIR), np.float32)
            for p in range(NPAIR):
                o = 400 * p
                we, wo = wc[2 * p], wc[2 * p + 1]
                o3, o4, o5, o6, o7, o8 = (o + 196, o + 230, o + 264, o + 298, o + 332, o + 366)
                # W1[98] pl0 rhs=xr0 -> rows (0, 32) = or_e
                wxa[:, o + 0] = we
                wxa[:, o + 32] = we
                # W2[98] pl1 rhs=xi0 -> rows (0, 32) = -oi_e
                wxa[:, o + 98] = -we
                wxa[:, o + 98 + 32] = -we
                # W3[34] pl0 rhs=xi0 -> rows (1, 33) = oi_e
                wxa[:, o3 + 1] = we
                wxa[:, o3 + 33] = we
                # W4[34] pl1 rhs=xr0 -> rows (1, 33) = or_e
                wxa[:, o4 + 1] = we
                wxa[:, o4 + 33] = we
                # W5[34] pl0 rhs=xr1 -> rows (64, 96) = or_o
                wxa[:, o5 + 0] = wo
                wxa[:, o5 + 32] = wo
                # W6[34] pl1 rhs=xi1 -> rows (64, 96) = -oi_o
                wxa[:, o6 + 0] = -wo
                wxa[:, o6 + 32] = -wo
                # W7[34] pl0 rhs=xi1 -> rows (65, 97) = oi_o
                wxa[:, o7 + 1] = wo
                wxa[:, o7 + 33] = wo
                # W8[34] pl1 rhs=xr1 -> rows (65, 97) = or_o
                wxa[:, o8 + 1] = wo
                wxa[:, o8 + 33] = wo
            in_maps.append(
                {
                    "input_real": np.ascontiguousarray(input_real[sl], dtype=np.float16),
                    "input_imag": np.ascontiguousarray(input_imag[sl], dtype=np.float16),
                    "wxa": np.ascontiguousarray(wxa, dtype=np.float16),
                }
            )
    else:
        xr16 = input_real.astype(np.float16).astype(np.float32)
        xi16 = input_imag.astype(np.float16).astype(np.float32)
        orv = np.einsum("btd,bt->bd", xr16, weight)
        oiv = np.einsum("btd,bt->bd", xi16, weight)
        for core in range(N_CORES):
            sl = slice(core * B_LOC, (core + 1) * B_LOC)
            oc_r = orv[sl].reshape(NPAIR, 2, D)
            oc_i = oiv[sl].reshape(NPAIR, 2, D)
            # [group, row, plane, pair, D]: groups (0,1) = even batch (dup),
            # (2,3) = odd batch (dup); rows = (or, oi) / (-oi, or)
            wm = np.empty((4, 2, 2, NPAIR, D), np.float32)
            for j in (0, 1):
                for g in (2 * j, 2 * j + 1):
                    wm[g, 0, 0] = oc_r[:, j]   # mv row 0 = or
                    wm[g, 1, 0] = oc_i[:, j]   # mv row 1 = oi
                    wm[g, 0, 1] = -oc_i[:, j]  # st' row 0 = -oi
                    wm[g, 1, 1] = oc_r[:, j]   # st' row 1 = or
            in_maps.append({"wm": np.ascontiguousarray(wm, dtype=np.float16)})
    return in_maps


def _expand_tri(tri, sym):
    """tri: [B, 128, 1280] packed block-upper rows -> full [B, D, D].
    Chunk m holds rows [128m,128m+128) x cols [128m, D). Lower blocks are
    mirrored (sym=+1) or negated-mirrored (sym=-1)."""
    Bn = tri.shape[0]
    full = np.empty((Bn, D, D), dtype=np.float32)
    for m in range(4):
        rs = slice(m * 128, (m + 1) * 128)
        full[:, rs, m * 128 :] = tri[:, :, TRI_OFF[m] : TRI_OFF[m] + D - m * 128]
    for m in range(4):
        for n in range(m):
            full[:, m * 128 : (m + 1) * 128, n * 128 : (n + 1) * 128] = (
                sym * full[:, n * 128 : (n + 1) * 128, m * 128 : (m + 1) * 128]
                .transpose(0, 2, 1)
            )
    return full


def run(input_real, input_imag, weight, trace=False, **spmd_kwargs):
    """Build+run; returns (out_r, out_i, BassKernelResults)."""
    from concourse.bass_utils import run_bass_kernel_spmd

    input_real = np.asarray(input_real, dtype=np.float32)
    input_imag = np.asarray(input_imag, dtype=np.float32)
    weight = np.asarray(weight, dtype=np.float32)
    assert input_real.shape == (B, T, D), input_real.shape
    assert weight.shape == (B, T), weight.shape

    nc = _get_nc()
    in_maps = _make_in_maps(input_real, input_imag, weight)
    res = run_bass_kernel_spmd(
        nc, in_maps, list(range(N_CORES)), trace=trace, **spmd_kwargs
    )
    tri_r = np.concatenate([np.asarray(r["out_r"]) for r in res.results], axis=0)
    tri_i = np.concatenate([np.asarray(r["out_i"]) for r in res.results], axis=0)
    out_r = _expand_tri(tri_r, sym=1.0)
    out_i = _expand_tri(tri_i, sym=-1.0)
    return out_r, out_i, res


def kernel(input_real, input_imag, weight):
    out_r, out_i, _ = run(input_real, input_imag, weight)
    return out_r, out_i
